# revision 1
# baseline (speedup 1.0000x reference)
"""Trainium2 Bass kernel for 3x3 same-padded conv (NCHW) scaled by 1/9.

Problem: x [32, 256, 56, 56] f32, w [256, 256, 3, 3] f32
         out = conv2d(x, w, padding=same) / 9    -> [32, 256, 56, 56] f32

Strategy:
  - Data-parallel over batch: 8 NeuronCores x 4 images each (SPMD, same program).
  - Per core: conv as 9 shifted matmuls. For each output chunk (8 rows x 56 cols
    = 448 spatial positions), accumulate 18 matmuls (9 taps x 2 ic-tiles of 128)
    into one PSUM bank: psum[oc=128, 448] += w_tap[ic=128, oc=128].T @
    x_shifted[ic=128, 448].
  - x is staged in SBUF as bf16 with a 1-pixel zero halo per image ([58 x 58])
    so every tap is a pure offset read. The halo is baked in on the HOST
    (zero-padded bf16 array) so every SBUF load is a fully contiguous DMA --
    6728 B per partition per descriptor instead of 112 B chunks, which
    measured ~16x faster. w is pre-transposed to [ic, oc] per tap on host
    (with the 1/9 folded in) and converted to bf16.
  - PSUM accumulates in fp32; DVE copies PSUM -> SBUF; DMA out fp32.
"""

import numpy as np
import ml_dtypes

import concourse.bacc as bacc
import concourse.mybir as mybir
import concourse.tile as tile
from concourse.bass_utils import run_bass_kernel_spmd

N_CORES = 8
N, IC, H, W = 32, 256, 56, 56
OC, KH, KW = 256, 3, 3
NPC = N // N_CORES          # images per core
ICT = IC // 128             # ic tiles
OCT = OC // 128             # oc tiles
HP, WP = H + 2, W + 2       # padded image
CHUNK_ROWS = 8              # output rows per PSUM tile
NCHUNK = H // CHUNK_ROWS    # 7
FREE = CHUNK_ROWS * W       # 448 <= 512 (one PSUM bank)

BF16 = mybir.dt.bfloat16
F32 = mybir.dt.float32

_compiled = None


def _build():
    nc = bacc.Bacc("TRN2", target_bir_lowering=False, debug=False,
                   num_devices=N_CORES)

    x_d = nc.dram_tensor("x", [NPC, ICT, 128, HP, WP], BF16,
                         kind="ExternalInput")
    w_d = nc.dram_tensor("wT", [ICT, 128, OCT, KH * KW, 128], BF16,
                         kind="ExternalInput")
    o_d = nc.dram_tensor("out", [NPC, OC, H, W], F32, kind="ExternalOutput")

    with tile.TileContext(nc) as tc:
        with (
            tc.tile_pool(name="xp", bufs=1) as xpool,
            tc.tile_pool(name="wp", bufs=1) as wpool,
            tc.tile_pool(name="op", bufs=4) as opool,
            tc.tile_pool(name="ps", bufs=8, space="PSUM") as pspool,
        ):
            # Spread input loads across the two HWDGE queue sets (SP +
            # Activation; gpsimd SWDGE measured too slow for the critical
            # path). w is split per oc-tile: the first matmuls need only the
            # oct0 half (0.3 MB/queue), so the real stream starts ~1 us
            # earlier than waiting for the full 0.59 MB w transfer.
            wsb = wpool.tile([128, ICT, OCT, KH * KW, 128], BF16)
            nc.sync.dma_start(wsb[:, 0, 0], w_d[0, :, 0])
            nc.scalar.dma_start(wsb[:, 1, 0], w_d[1, :, 0])

            # Padded x tiles, one per (image, ic-tile); halo pre-baked on host
            # so loads are fully contiguous. img0 is loaded as row-halves with
            # the top halves first on both queues, so chunk-0 matmuls can
            # start after ~1.2 MB instead of the full 2.3 MB.
            xtiles = {}
            for img in range(NPC):
                for ict in range(ICT):
                    xt = xpool.tile([128, HP, WP], BF16, tag=f"x{img}_{ict}",
                                    name=f"x{img}_{ict}")
                    xtiles[(img, ict)] = xt
            # img0 arrives in 4 row-pieces per ic-tile so chunk-0 matmuls can
            # start after ~0.5 MB lands rather than the full 2.3 MB.
            cuts = [0, 16, 30, 44, HP]
            for i, (lo, hi) in enumerate(zip(cuts, cuts[1:])):
                nc.sync.dma_start(xtiles[(0, 0)][:, lo:hi], x_d[0, 0, :, lo:hi])
                nc.scalar.dma_start(xtiles[(0, 1)][:, lo:hi],
                                    x_d[0, 1, :, lo:hi])
                if i == 1:
                    # oct1 weights after the first two img0 pieces; needed
                    # only ~24 us into the stream.
                    nc.sync.dma_start(wsb[:, 0, 1], w_d[0, :, 1])
                    nc.scalar.dma_start(wsb[:, 1, 1], w_d[1, :, 1])
            for img in range(1, NPC):
                for ict in range(ICT):
                    eng = nc.sync if (img * ICT + ict) % 2 == 0 else nc.scalar
                    eng.dma_start(xtiles[(img, ict)][:], x_d[img, ict])

            # PE pre-warm: dummy matmuls while the first DMAs are in flight so
            # the HAM clock gate is already at 8/8 when the real stream starts.
            zs = wpool.tile([128, 512], BF16, name="zs")
            nc.gpsimd.memset(zs[:], 0.0)
            zp = pspool.tile([128, 512], F32, tag="pt", name="zp")
            for _ in range(18):
                nc.tensor.matmul(zp[:], zs[:, :128], zs[:], start=True,
                                 stop=True)

            for img in range(NPC):
                for oct_ in range(OCT):
                    for chunk in range(NCHUNK):
                        y0 = chunk * CHUNK_ROWS
                        pt = pspool.tile([128, CHUNK_ROWS, W], F32, tag="pt",
                                         name=f"pt{img}_{oct_}_{chunk}")
                        mm = 0
                        for tap in range(KH * KW):
                            for ict in range(ICT):
                                dy, dx = tap // 3, tap % 3
                                xt = xtiles[(img, ict)]
                                rhs = xt[:, y0 + dy:y0 + dy + CHUNK_ROWS,
                                         dx:dx + W]
                                lhsT = wsb[:, ict, oct_, tap, :]
                                nc.tensor.matmul(
                                    pt[:], lhsT, rhs,
                                    start=(mm == 0),
                                    stop=(mm == KH * KW * ICT - 1),
                                )
                                mm += 1
                        ot = opool.tile([128, CHUNK_ROWS, W], F32, tag="ot",
                                        name=f"ot{img}_{oct_}_{chunk}")
                        nc.vector.tensor_copy(ot[:], pt[:])
                        out_eng = nc.sync if chunk % 2 == 0 else nc.scalar
                        out_eng.dma_start(
                            o_d[img, oct_ * 128:(oct_ + 1) * 128,
                                y0:y0 + CHUNK_ROWS, :],
                            ot[:])

    nc.compile()
    return nc


def _get_compiled():
    global _compiled
    if _compiled is None:
        _compiled = _build()
    return _compiled


def _prep_inputs(x, w):
    bf16 = ml_dtypes.bfloat16
    # [oc, ic, kh, kw] -> [ict, ic_in, tap, oct, oc_in], with 1/9 folded in
    # [oc, ic, kh, kw] -> [ict, ic_in, oct, tap, oc_in], 1/9 folded in
    wT = np.ascontiguousarray(
        (w.astype(np.float32) / (KH * KW)).transpose(1, 2, 3, 0)
        .reshape(ICT, 128, KH * KW, OCT, 128).transpose(0, 1, 3, 2, 4)
    ).astype(bf16)
    # Host-side zero-padded bf16 x: [N, ICT, 128, HP, WP]
    xp = np.zeros((N, ICT, 128, HP, WP), dtype=bf16)
    xp[:, :, :, 1:H + 1, 1:W + 1] = x.reshape(N, ICT, 128, H, W).astype(bf16)
    return [
        {"x": xp[c * NPC:(c + 1) * NPC], "wT": wT}
        for c in range(N_CORES)
    ]


def kernel(x, w, _trace=False, _trace_kwargs=None):
    nc = _get_compiled()
    in_maps = _prep_inputs(np.asarray(x), np.asarray(w))
    res = run_bass_kernel_spmd(nc, in_maps, list(range(N_CORES)),
                               trace=_trace, **(_trace_kwargs or {}))
    out = np.concatenate([res.results[c]["out"] for c in range(N_CORES)],
                         axis=0)
    if _trace:
        return out, res
    return out

